# revision 55
# baseline (speedup 1.0000x reference)
"""Single-head attention (B=4, N=2048, D=1024) on 8 Trainium2 NeuronCores.

Sharding: core c handles batch c//2 and KEY half c%2.  scores = q @ k^T is
rewritten as x @ G @ x_k^T with G = Wq^T Wk folded on the HOST (free), so the
kernel never computes q or k:

    u  = G x_k^T          (128 MMs, replaces K-proj 128 + Q-proj 256)
    s  = u^T-contract-x   (256 MMs)
    v  = x_k Wv^T         (128 MMs)
    o  = exp(s*scale) v   (256 MMs + 128 tiny denominator MMs)

768 big matmuls/core vs 1024 for the direct form.  Each core emits partial
(unnormalized) output + softmax denominator over its key half; the host
combines: out = (oA + oB) / (dA + dB).

Per-core inputs are pre-rotated on the host so the core's key half is always
columns [0:1024) (odd cores' query order is rolled; host un-rolls the output
rows).  All host-side tensors are laid out [partition, chunk, col] contiguous
so every input DMA is a single fully-contiguous transfer, issued in
consumption order.

All matmuls bf16 with fp32 PSUM accumulation; exp in fp32 on the scalar
engine.  Unnormalized softmax (no max subtraction) is safe: |scores/sqrt(D)|
is ~N(0, 0.33^2) for these inputs.  Partial outputs stored bf16 (rel err
~6e-3 vs the 2e-2 budget).
"""

from contextlib import ExitStack

import ml_dtypes
import numpy as np

import concourse.bass as bass
import concourse.mybir as mybir
import concourse.tile as tile
from concourse.bass_utils import run_bass_kernel_spmd

B, N, D = 4, 2048, 1024
NCORES = 8
P = 128
NQ = N            # queries per core (full batch)
NKH = N // 2      # keys per core (half)
DC = D // P       # 8 contraction chunks
EC = D // P       # 8 embed blocks
JB = NKH // P     # 8 key blocks
F = 512           # matmul moving free dim (one PSUM bank of fp32)
SCALE = 1.0 / np.sqrt(D)

BF = mybir.dt.bfloat16
F32 = mybir.dt.float32


def _attention_kernel(ctx, tc, out, ins):
    nc = tc.nc

    consts = ctx.enter_context(tc.tile_pool(name="consts", bufs=1))
    psmain = ctx.enter_context(tc.tile_pool(name="psmain", bufs=3, space="PSUM"))
    psav = ctx.enter_context(tc.tile_pool(name="psav", bufs=5, space="PSUM"))
    outp = ctx.enter_context(tc.tile_pool(name="outp", bufs=2))
    small = ctx.enter_context(tc.tile_pool(name="small", bufs=2))

    # Resident SBUF tensors (~130KB/partition).  "head" packs the first
    # uT tile's whole working set (xk cols 0:512 + wu e0-block) into one
    # tensor so a single first DMA gates the first real matmul.
    names = ["head", "xk1", "xq0", "xq1", "wu0", "wu1", "wv0", "wv1"]
    widths = {"head": F + P, "wu0": 3 * P}
    sb = {
        n: consts.tile([P, DC, widths.get(n, F)], BF, tag=n, name=n)
        for n in names
    }
    sb["xk0"] = sb["head"][:, :, 0:F]
    uT_sb = consts.tile([P, EC, NKH], BF, tag="uT")      # [p, e-blk, key]
    v_sb = consts.tile([P, JB, D], BF, tag="v")          # [p, key-blk, e]
    pT_sb = consts.tile([P, JB, NQ], BF, tag="pT")       # [p, key-blk, query]
    ones_sb = consts.tile([P, 1], BF, tag="ones")

    nc.vector.memset(ones_sb, 1.0)

    # Input DMAs: fully-contiguous transfers in consumption order.  The
    # first transfer has a ~5us completion-latency floor regardless of
    # size, so the first tile's whole working set rides in one transfer.
    dma = {}
    for n in ("head", "wu0", "wu1", "xk1", "wv0", "wv1", "xq0", "xq1"):
        dma[n] = nc.sync.dma_start(out=sb[n], in_=ins[n])
    in_dmas = list(dma.values())

    def sp_observe(inst, why):
        # One-wait nops on the sync sequencer: make SP observe a proc's sem
        # tick so later SP instructions (the kernel-tail drain) don't need
        # to aggregate multiple sync waits (HW allows one per instruction).
        n = nc.sync.nop(hint="observe")
        tile.add_dep_helper(n.ins, inst.ins, reason=why)

    # HAM pre-warm: keep the PE busy on dummy matmuls over zeroed SBUF while
    # the first input DMAs are still on the wire (first pair lands ~15us:
    # ~8us framework preamble + ~6us wire), so the PE clock-gate (needs
    # ~3.4us of sustained activity) opens before real work.  warm_ps is
    # psav slot 0 and is never read: touch matmuls can target its corner
    # with no WAR dep, keeping each touch to a single DMA wait.
    warm_src = small.tile([P, 640], BF, tag="warm")
    nc.vector.memset(warm_src, 0.0)
    warm_ps = psav.tile([P, F], F32, tag="po")
    N_WARM = 17

    def warm(n):
        for _ in range(n):
            nc.tensor.matmul(
                warm_ps,
                lhsT=warm_src[:, 0:P],
                rhs=warm_src[:, P : P + F],
                start=True,
                stop=True,
            )

    warm(N_WARM)

    def touch(dname, tname, col=0):
        # Attach one input DMA's wait to a trivial matmul into the warm-up
        # PSUM corner (WAW vs earlier PE writes only — same engine, free),
        # so no later real matmul needs a DMA wait on top of its PSUM-WAR
        # or RAW wait (PE matmuls support a single sync-wait command).
        t = sb[tname]
        mm = nc.tensor.matmul(
            warm_ps[0:1, 0:1], lhsT=t[:, 0, col : col + 1],
            rhs=t[:, 0, col : col + 1], start=True, stop=True,
        )
        tile.add_dep_helper(mm.ins, dma[dname].ins, reason="touch DMA on PE")

    ktiles = [sb["xk0"], sb["xk1"]]
    qtiles = [sb["xk0"], sb["xk1"], sb["xq0"], sb["xq1"]]
    wvtiles = [sb["wv0"], sb["wv1"]]

    def wu_sl(e, c):
        # e0 rides in the head tensor; e1-3 in wu0 (3 blocks); e4-7 in wu1.
        if e == 0:
            return sb["head"][:, c, F : F + P]
        if e < 4:
            return sb["wu0"][:, c, (e - 1) * P : e * P]
        return sb["wu1"][:, c, (e - 4) * P : (e - 3) * P]

    # Phase 1a: uT[e, j] = sum_d G[e, d] xk[j, d] — lhsT = wuT[d, e-blk],
    # rhs = xk[d, j-tile].  (wuT[d, e] = G[e, d] is host-folded Wk^T Wq.)
    touch("head", "head")
    for jt in range(2):
        for e in range(EC):
            ps = psmain.tile([P, F], F32, tag="ps")
            for c in range(DC):
                nc.tensor.matmul(
                    ps,
                    lhsT=wu_sl(e, c),
                    rhs=ktiles[jt][:, c, :],
                    start=(c == 0),
                    stop=(c == DC - 1),
                )
            nc.vector.tensor_copy(out=uT_sb[:, e, jt * F : (jt + 1) * F], in_=ps)
            if jt == 0 and e == 0:
                touch("wu0", "wu0")
            if jt == 0 and e == 3:
                touch("wu1", "wu1")
        if jt == 0:
            touch("xk1", "xk1")

    # Phase 1b: v[j, e] — lhsT = xk[d, j-blk], rhs = WvT[d, e-tile]
    touch("wv0", "wv0")
    touch("wv1", "wv1")
    for j in range(JB):
        for et in range(D // F):
            ps = psmain.tile([P, F], F32, tag="ps")
            for c in range(DC):
                nc.tensor.matmul(
                    ps,
                    lhsT=ktiles[j // 4][:, c, (j % 4) * P : (j % 4 + 1) * P],
                    rhs=wvtiles[et][:, c, :],
                    start=(c == 0),
                    stop=(c == DC - 1),
                )
            nc.vector.tensor_copy(out=v_sb[:, j, et * F : (et + 1) * F], in_=ps)

    # Phase 2b: scoresT[j, i] = sum_e uT[e, j] x[i, e], p = exp(s*SCALE).
    # j outer / it inner so every ACT tick a later 2c group needs (except
    # the j=7 tile of its it-group) is observed by the PE before 2c starts.
    touch("xq0", "xq0")
    touch("xq1", "xq1")
    for j in range(JB):
        for it in range(NQ // F):
            ps = psmain.tile([P, F], F32, tag="ps")
            for e in range(EC):
                nc.tensor.matmul(
                    ps,
                    lhsT=uT_sb[:, e, j * P : (j + 1) * P],
                    rhs=qtiles[it][:, e, :],
                    start=(e == 0),
                    stop=(e == EC - 1),
                )
            last_exp = nc.scalar.activation(
                out=pT_sb[:, j, it * F : (it + 1) * F],
                in_=ps,
                func=mybir.ActivationFunctionType.Exp,
                scale=float(SCALE),
            )

    for d in in_dmas:
        sp_observe(d, "observe input DMA on SP")

    # Phase 2c: partial out[i, 0:1024] = pT.T @ v, partial denom in column
    # 1024.  16 SWDGE stores via GpSimd (the hardware-DMA sem pool is
    # exhausted by the inputs; a sync-engine store would need a recycled
    # sem's reuse-guard wait on top of its data wait -> walrus error).
    outr = out.rearrange("(g p) e -> g p e", p=P)
    oguard = small.tile([P, NQ // P], F32, tag="oguard")
    gguard = small.tile([P, NQ // P + 2], F32, tag="gguard")
    out_dmas = []
    for ib in range(NQ // P):
        o_sb = outp.tile([P, D + 1], BF, tag="o")
        g = None
        if ib >= 2:
            # Pre-observe the output-DMA tick (WAR on o_sb slot reuse) on
            # the DVE so the copies below carry only their one data wait.
            g = nc.vector.memset(oguard[0:1, ib : ib + 1], 0.0)
            tile.add_dep_helper(
                g.ins, out_dmas[ib - 2].ins, reason="observe out DMA on DVE"
            )
        # Absorb the WAW against the slot's previous DVE writes in a guard
        # write of its own, so the data copies keep a single wait each.
        g2 = nc.vector.memset(o_sb[0:1, 0:1], 0.0)
        if g is not None:
            tile.add_dep_helper(g2.ins, g.ins, False, reason="order after oguard")
        # pd first: its slot's previous reader (dcp two groups back) is the
        # earliest of the old group's copies, keeping every leader's WAR at
        # least one full group in the past on the 5-slot rotation.
        pd = psav.tile([P, F], F32, tag="po")
        po0 = psav.tile([P, F], F32, tag="po")
        po1 = psav.tile([P, F], F32, tag="po")
        if ib == NQ // P - 1:
            # Final group: all po1 matmuls first, so its copy and store
            # half launch ~1.7us before the group ends and the framework's
            # DVE pipeline-drain insertion lands mid-group (PE still busy)
            # instead of on the critical tail.
            for j in range(JB):
                nc.tensor.matmul(
                    po1, lhsT=pT_sb[:, j, ib * P : (ib + 1) * P],
                    rhs=v_sb[:, j, F : 2 * F],
                    start=(j == 0), stop=(j == JB - 1),
                )
            for j in range(JB):
                lhsT = pT_sb[:, j, ib * P : (ib + 1) * P]
                nc.tensor.matmul(
                    po0, lhsT=lhsT, rhs=v_sb[:, j, 0:F],
                    start=(j == 0), stop=(j == JB - 1),
                )
                last_mm = nc.tensor.matmul(
                    pd[:, 0:1], lhsT=lhsT, rhs=ones_sb,
                    start=(j == 0), stop=(j == JB - 1),
                )
        else:
            for j in range(JB):
                lhsT = pT_sb[:, j, ib * P : (ib + 1) * P]
                nc.tensor.matmul(
                    po0, lhsT=lhsT, rhs=v_sb[:, j, 0:F],
                    start=(j == 0), stop=(j == JB - 1),
                )
                nc.tensor.matmul(
                    po1, lhsT=lhsT, rhs=v_sb[:, j, F : 2 * F],
                    start=(j == 0), stop=(j == JB - 1),
                )
                last_mm = nc.tensor.matmul(
                    pd[:, 0:1], lhsT=lhsT, rhs=ones_sb,
                    start=(j == 0), stop=(j == JB - 1),
                )
        # Output layout per row: [po0 (0:F) | den (F) | po1 (F+1:D+1)] —
        # the denominator sits between the halves so the final group's
        # split stores each depend on a single copy.
        if ib == NQ // P - 1:
            # Kernel-critical tail: c1 waits only po1's stop (the group's
            # 2nd-to-last PE tick), so its store half ships ~0.2us before
            # the denominator matmul even retires; the other half follows
            # its own copy.  High priority places the copies ahead of the
            # teardown drains on the DVE queue.  The split stores are
            # SWDGE descriptors 15 and 16, recycling the 8-sem pool's
            # sems of stores 7 and 8 — pre-observed on GpSimd so each
            # dma_start keeps a single data-ready wait.
            gg_b = nc.gpsimd.memset(gguard[0:1, ib + 1 : ib + 2], 0.0)
            tile.add_dep_helper(
                gg_b.ins, out_dmas[7].ins, reason="observe sem reuse on GpSimd"
            )
            with tc.high_priority():
                c1 = nc.vector.tensor_copy(out=o_sb[:, F + 1 : D + 1], in_=po1)
                tile.add_dep_helper(c1.ins, g2.ins, False, reason="order after g2")
            st_b = nc.gpsimd.dma_start(
                out=outr[ib][:, F + 1 : D + 1], in_=o_sb[:, F + 1 : D + 1]
            )
            gg_a = nc.gpsimd.memset(gguard[0:1, ib + 2 : ib + 3], 0.0)
            tile.add_dep_helper(
                gg_a.ins, out_dmas[8].ins, reason="observe sem reuse on GpSimd"
            )
            with tc.high_priority():
                dcp = nc.vector.tensor_copy(out=o_sb[:, F : F + 1], in_=pd[:, 0:1])
                tile.add_dep_helper(dcp.ins, c1.ins, False, reason="order after c1")
                c0 = nc.vector.tensor_copy(out=o_sb[:, 0:F], in_=po0)
                tile.add_dep_helper(c0.ins, dcp.ins, False, reason="order after dcp")
            st_a = nc.gpsimd.dma_start(
                out=outr[ib][:, 0 : F + 1], in_=o_sb[:, 0 : F + 1]
            )
            out_dmas.extend([st_b, st_a])
            last_cp = c0
            continue
        # Denominator copy first: pd's stop-matmul is the group's last
        # PE tick, so this copy's PE wait covers po0/po1 and the po
        # copies need only their (buffer-reuse) DVE wait.  The explicit
        # sync=False deps pin the scheduler to that order.
        dcp = nc.vector.tensor_copy(out=o_sb[:, F : F + 1], in_=pd[:, 0:1])
        tile.add_dep_helper(dcp.ins, g2.ins, False, reason="order after guard")
        c0 = nc.vector.tensor_copy(out=o_sb[:, 0:F], in_=po0)
        tile.add_dep_helper(c0.ins, dcp.ins, False, reason="order after dcp")
        last_cp = nc.vector.tensor_copy(out=o_sb[:, F + 1 : D + 1], in_=po1)
        tile.add_dep_helper(last_cp.ins, c0.ins, False, reason="order after c0")
        if ib >= 8:
            # SWDGE queue lap 2: pre-observe the queue's previous store on
            # GpSimd so the dma_start keeps its single data-ready wait.
            gg = nc.gpsimd.memset(gguard[0:1, ib : ib + 1], 0.0)
            tile.add_dep_helper(
                gg.ins, out_dmas[ib - 8].ins, reason="observe queue lap on GpSimd"
            )
        out_dmas.append(nc.gpsimd.dma_start(out=outr[ib], in_=o_sb))

    # Let SP observe every remaining proc's final tick so the auto-generated
    # kernel-tail drain needs no aggregated multi-sem wait of its own.
    for dd in out_dmas:
        sp_observe(dd, "observe output DMA on SP")
    sp_observe(last_exp, "observe ACT on SP")
    sp_observe(last_mm, "observe PE on SP")
    sp_observe(last_cp, "observe DVE on SP")


def build_attention_module():
    nc = bass.Bass(trn_type="TRN2", target_bir_lowering=False, debug=False)
    sizes = {
        "head": F + P, "wu0": 3 * P,
        "wu1": F, "xk1": F, "wv0": F, "wv1": F, "xq0": F, "xq1": F,
    }
    ins = {
        n: nc.dram_tensor(n, [P, DC * w], BF, kind="ExternalInput").ap()
        for n, w in sizes.items()
    }
    out = nc.dram_tensor("out", [NQ, D + 1], BF, kind="ExternalOutput").ap()
    with tile.TileContext(nc) as tc:
        with ExitStack() as ctx:
            _attention_kernel(ctx, tc, out, ins)
    return nc


_module_cache = None


def _get_module():
    global _module_cache
    if _module_cache is None:
        _module_cache = build_attention_module()
    return _module_cache


def _pc(a):
    """[D, M] -> [P, DC, M] (partition, chunk, col) view."""
    return a.reshape(DC, P, a.shape[1]).transpose(1, 0, 2)


def _pcf(a, lo, hi):
    """[D, M] fp32 -> [P, DC*(hi-lo)] bf16, [p, c, col] contiguous slice."""
    return (
        np.ascontiguousarray(_pc(a)[:, :, lo:hi])
        .reshape(P, DC * (hi - lo))
        .astype(ml_dtypes.bfloat16)
    )


def make_in_maps(x, Wq, Wk, Wv):
    x = np.asarray(x, dtype=np.float32)
    Wq = np.asarray(Wq, dtype=np.float32)
    Wk = np.asarray(Wk, dtype=np.float32)
    Wv = np.asarray(Wv, dtype=np.float32)
    # scores = q k^T = x (Wq^T Wk) x_k^T; fold G on the host.  The kernel's
    # stationary operand is wuT[d, e] = G[e, d] = (Wk^T Wq)[d, e].
    wu = Wk.T @ Wq
    wvT = np.ascontiguousarray(Wv.T)
    shared = {
        "wu0": _pcf(wu, P, 4 * P), "wu1": _pcf(wu, F, 2 * F),
        "wv0": _pcf(wvT, 0, F), "wv1": _pcf(wvT, F, 2 * F),
    }
    wu_e0 = _pc(wu)[:, :, 0:P]  # rides in the head transfer
    in_maps = []
    for core in range(NCORES):
        b, half = divmod(core, 2)
        xb = x[b]
        if half:
            xb = np.concatenate([xb[NKH:], xb[:NKH]], axis=0)
        xt = np.ascontiguousarray(xb.T)  # [D, N], key half first
        xk = xt[:, 0:NKH]
        xq = xt[:, NKH:]
        head = (
            np.concatenate([_pc(xk)[:, :, 0:F], wu_e0], axis=2)
            .reshape(P, DC * (F + P))
            .astype(ml_dtypes.bfloat16)
        )
        in_maps.append(
            {
                "head": head, "xk1": _pcf(xk, F, 2 * F),
                "xq0": _pcf(xq, 0, F), "xq1": _pcf(xq, F, 2 * F),
                **shared,
            }
        )
    return in_maps


def _install_ntff_hook_shim():
    """The container's `antenv` stub lacks axon_hooks; register an equivalent
    built on trn_agent_boot's ctypes NTFF driver so trace=True works."""
    import sys
    import types

    if "antenv.axon_hooks" in sys.modules:
        return
    try:
        from trn_agent_boot.trn_boot import _ntff_profile_via_ctypes

        hook = _ntff_profile_via_ctypes("/opt/axon/libaxon_pjrt.so")
    except Exception:
        hook = None
    mod = types.ModuleType("antenv.axon_hooks")
    mod.get_axon_ntff_profile_hook = lambda: hook
    sys.modules["antenv.axon_hooks"] = mod


def kernel(x, Wq, Wk, Wv, _trace=False, _trace_cores=None):
    if _trace:
        _install_ntff_hook_shim()
    in_maps = make_in_maps(x, Wq, Wk, Wv)
    nc = _get_module()
    res = run_bass_kernel_spmd(
        nc,
        in_maps,
        core_ids=list(range(NCORES)),
        trace=_trace,
        trace_cores=_trace_cores,
    )
    out = np.empty((B, N, D), dtype=np.float32)
    for b in range(B):
        r0 = np.asarray(res.results[2 * b]["out"], dtype=np.float32)
        r1 = np.asarray(res.results[2 * b + 1]["out"], dtype=np.float32)
        r1 = np.roll(r1, NKH, axis=0)  # undo the odd-core query rotation
        osum = r0 + r1
        den = osum[:, F : F + 1]  # denominator column sits between the halves
        out[b, :, 0:F] = osum[:, 0:F] / den
        out[b, :, F:D] = osum[:, F + 1 : D + 1] / den
    if _trace:
        return out, res
    return out


# revision 56
# speedup vs baseline: 1.1881x; 1.1881x over previous
"""Single-head attention (B=4, N=2048, D=1024) on 8 Trainium2 NeuronCores.

Sharding: core c handles batch c//2 and KEY half c%2.  scores = q @ k^T is
rewritten as x @ G @ x_k^T with G = Wq^T Wk folded on the HOST (free), so the
kernel never computes q or k:

    u  = G x_k^T          (128 MMs, replaces K-proj 128 + Q-proj 256)
    s  = u^T-contract-x   (256 MMs)
    v  = x_k Wv^T         (128 MMs)
    o  = exp(s*scale) v   (256 MMs + 128 tiny denominator MMs)

768 big matmuls/core vs 1024 for the direct form.  Each core emits partial
(unnormalized) output + softmax denominator over its key half; the host
combines: out = (oA + oB) / (dA + dB).

Per-core inputs are pre-rotated on the host so the core's key half is always
columns [0:1024) (odd cores' query order is rolled; host un-rolls the output
rows).  All host-side tensors are laid out [partition, chunk, col] contiguous
so every input DMA is a single fully-contiguous transfer, issued in
consumption order.

All matmuls bf16 with fp32 PSUM accumulation; exp in fp32 on the scalar
engine.  Unnormalized softmax (no max subtraction) is safe: |scores/sqrt(D)|
is ~N(0, 0.33^2) for these inputs.  Partial outputs stored bf16 (rel err
~6e-3 vs the 2e-2 budget).
"""

from contextlib import ExitStack

import ml_dtypes
import numpy as np

import concourse.bass as bass
import concourse.mybir as mybir
import concourse.tile as tile
from concourse.bass_utils import run_bass_kernel_spmd

B, N, D = 4, 2048, 1024
NCORES = 8
P = 128
NQ = N            # queries per core (full batch)
NKH = N // 2      # keys per core (half)
DC = D // P       # 8 contraction chunks
EC = D // P       # 8 embed blocks
JB = NKH // P     # 8 key blocks
F = 512           # matmul moving free dim (one PSUM bank of fp32)
SCALE = 1.0 / np.sqrt(D)

BF = mybir.dt.bfloat16
F32 = mybir.dt.float32


def _attention_kernel(ctx, tc, out, ins):
    nc = tc.nc

    consts = ctx.enter_context(tc.tile_pool(name="consts", bufs=1))
    psmain = ctx.enter_context(tc.tile_pool(name="psmain", bufs=3, space="PSUM"))
    psav = ctx.enter_context(tc.tile_pool(name="psav", bufs=5, space="PSUM"))
    outp = ctx.enter_context(tc.tile_pool(name="outp", bufs=2))
    small = ctx.enter_context(tc.tile_pool(name="small", bufs=2))

    # Resident SBUF tensors (~130KB/partition).  "head" packs the first
    # uT tile's whole working set (xk cols 0:512 + wu e0-block) into one
    # tensor so a single first DMA gates the first real matmul.
    names = ["head", "xk1", "xq0", "xq1", "wu0", "wu1", "wv0", "wv1"]
    widths = {"head": F + P, "wu0": 3 * P}
    sb = {
        n: consts.tile([P, DC, widths.get(n, F)], BF, tag=n, name=n)
        for n in names
    }
    sb["xk0"] = sb["head"][:, :, 0:F]
    uT_sb = consts.tile([P, EC, NKH], BF, tag="uT")      # [p, e-blk, key]
    v_sb = consts.tile([P, JB, D], BF, tag="v")          # [p, key-blk, e]
    pT_sb = consts.tile([P, JB, NQ], BF, tag="pT")       # [p, key-blk, query]
    ones_sb = consts.tile([P, 1], BF, tag="ones")

    nc.vector.memset(ones_sb, 1.0)

    # Input DMAs: fully-contiguous transfers in consumption order.  The
    # first transfer has a ~5us completion-latency floor regardless of
    # size, so the first tile's whole working set rides in one transfer.
    dma = {}
    for n in ("head", "wu0", "wu1", "xk1", "wv0", "wv1", "xq0", "xq1"):
        dma[n] = nc.sync.dma_start(out=sb[n], in_=ins[n])
    in_dmas = list(dma.values())

    def sp_observe(inst, why):
        # One-wait nops on the sync sequencer: make SP observe a proc's sem
        # tick so later SP instructions (the kernel-tail drain) don't need
        # to aggregate multiple sync waits (HW allows one per instruction).
        n = nc.sync.nop(hint="observe")
        tile.add_dep_helper(n.ins, inst.ins, reason=why)

    # HAM pre-warm: keep the PE busy on dummy matmuls over zeroed SBUF while
    # the first input DMAs are still on the wire (first pair lands ~15us:
    # ~8us framework preamble + ~6us wire), so the PE clock-gate (needs
    # ~3.4us of sustained activity) opens before real work.  warm_ps is
    # psav slot 0 and is never read: touch matmuls can target its corner
    # with no WAR dep, keeping each touch to a single DMA wait.
    warm_src = small.tile([P, 640], BF, tag="warm")
    nc.vector.memset(warm_src, 0.0)
    warm_ps = psav.tile([P, F], F32, tag="po")
    N_WARM = 17

    def warm(n):
        for _ in range(n):
            nc.tensor.matmul(
                warm_ps,
                lhsT=warm_src[:, 0:P],
                rhs=warm_src[:, P : P + F],
                start=True,
                stop=True,
            )

    warm(N_WARM)

    def touch(dname, tname, col=0):
        # Attach one input DMA's wait to a trivial matmul into the warm-up
        # PSUM corner (WAW vs earlier PE writes only — same engine, free),
        # so no later real matmul needs a DMA wait on top of its PSUM-WAR
        # or RAW wait (PE matmuls support a single sync-wait command).
        t = sb[tname]
        mm = nc.tensor.matmul(
            warm_ps[0:1, 0:1], lhsT=t[:, 0, col : col + 1],
            rhs=t[:, 0, col : col + 1], start=True, stop=True,
        )
        tile.add_dep_helper(mm.ins, dma[dname].ins, reason="touch DMA on PE")

    ktiles = [sb["xk0"], sb["xk1"]]
    qtiles = [sb["xk0"], sb["xk1"], sb["xq0"], sb["xq1"]]
    wvtiles = [sb["wv0"], sb["wv1"]]

    def wu_sl(e, c):
        # e0 rides in the head tensor; e1-3 in wu0 (3 blocks); e4-7 in wu1.
        if e == 0:
            return sb["head"][:, c, F : F + P]
        if e < 4:
            return sb["wu0"][:, c, (e - 1) * P : e * P]
        return sb["wu1"][:, c, (e - 4) * P : (e - 3) * P]

    # Phase 1a: uT[e, j] = sum_d G[e, d] xk[j, d] — lhsT = wuT[d, e-blk],
    # rhs = xk[d, j-tile].  (wuT[d, e] = G[e, d] is host-folded Wk^T Wq.)
    touch("head", "head")
    for jt in range(2):
        for e in range(EC):
            ps = psmain.tile([P, F], F32, tag="ps")
            for c in range(DC):
                nc.tensor.matmul(
                    ps,
                    lhsT=wu_sl(e, c),
                    rhs=ktiles[jt][:, c, :],
                    start=(c == 0),
                    stop=(c == DC - 1),
                )
            nc.vector.tensor_copy(out=uT_sb[:, e, jt * F : (jt + 1) * F], in_=ps)
            if jt == 0 and e == 0:
                touch("wu0", "wu0")
            if jt == 0 and e == 3:
                touch("wu1", "wu1")
        if jt == 0:
            touch("xk1", "xk1")

    # Phase 1b: v[j, e] — lhsT = xk[d, j-blk], rhs = WvT[d, e-tile]
    touch("wv0", "wv0")
    touch("wv1", "wv1")
    for j in range(JB):
        for et in range(D // F):
            ps = psmain.tile([P, F], F32, tag="ps")
            for c in range(DC):
                nc.tensor.matmul(
                    ps,
                    lhsT=ktiles[j // 4][:, c, (j % 4) * P : (j % 4 + 1) * P],
                    rhs=wvtiles[et][:, c, :],
                    start=(c == 0),
                    stop=(c == DC - 1),
                )
            nc.vector.tensor_copy(out=v_sb[:, j, et * F : (et + 1) * F], in_=ps)

    # Phase 2b: scoresT[j, i] = sum_e uT[e, j] x[i, e], p = exp(s*SCALE).
    # j outer / it inner so every ACT tick a later 2c group needs (except
    # the j=7 tile of its it-group) is observed by the PE before 2c starts.
    touch("xq0", "xq0")
    touch("xq1", "xq1")
    for j in range(JB):
        for it in range(NQ // F):
            ps = psmain.tile([P, F], F32, tag="ps")
            for e in range(EC):
                nc.tensor.matmul(
                    ps,
                    lhsT=uT_sb[:, e, j * P : (j + 1) * P],
                    rhs=qtiles[it][:, e, :],
                    start=(e == 0),
                    stop=(e == EC - 1),
                )
            last_exp = nc.scalar.activation(
                out=pT_sb[:, j, it * F : (it + 1) * F],
                in_=ps,
                func=mybir.ActivationFunctionType.Exp,
                scale=float(SCALE),
            )

    for d in in_dmas:
        sp_observe(d, "observe input DMA on SP")

    # Phase 2c: partial out[i, 0:1024] = pT.T @ v, partial denom in column
    # 1024.  16 SWDGE stores via GpSimd (the hardware-DMA sem pool is
    # exhausted by the inputs; a sync-engine store would need a recycled
    # sem's reuse-guard wait on top of its data wait -> walrus error).
    outr = out.rearrange("(g p) e -> g p e", p=P)
    oguard = small.tile([P, NQ // P], F32, tag="oguard")
    gguard = small.tile([P, NQ // P + 2], F32, tag="gguard")
    out_dmas = []
    for ib in range(NQ // P):
        o_sb = outp.tile([P, D + 1], BF, tag="o")
        g = None
        if ib >= 2:
            # Pre-observe the output-DMA tick (WAR on o_sb slot reuse) on
            # the DVE so the copies below carry only their one data wait.
            g = nc.vector.memset(oguard[0:1, ib : ib + 1], 0.0)
            tile.add_dep_helper(
                g.ins, out_dmas[ib - 2].ins, reason="observe out DMA on DVE"
            )
        # Absorb the WAW against the slot's previous DVE writes in a guard
        # write of its own, so the data copies keep a single wait each.
        g2 = nc.vector.memset(o_sb[0:1, 0:1], 0.0)
        if g is not None:
            tile.add_dep_helper(g2.ins, g.ins, False, reason="order after oguard")
        # pd first: its slot's previous reader (dcp two groups back) is the
        # earliest of the old group's copies, keeping every leader's WAR at
        # least one full group in the past on the 5-slot rotation.
        pd = psav.tile([P, F], F32, tag="po")
        po0 = psav.tile([P, F], F32, tag="po")
        po1 = psav.tile([P, F], F32, tag="po")
        for j in range(JB):
            lhsT = pT_sb[:, j, ib * P : (ib + 1) * P]
            nc.tensor.matmul(
                po0, lhsT=lhsT, rhs=v_sb[:, j, 0:F],
                start=(j == 0), stop=(j == JB - 1),
            )
            nc.tensor.matmul(
                po1, lhsT=lhsT, rhs=v_sb[:, j, F : 2 * F],
                start=(j == 0), stop=(j == JB - 1),
            )
            last_mm = nc.tensor.matmul(
                pd[:, 0:1], lhsT=lhsT, rhs=ones_sb,
                start=(j == 0), stop=(j == JB - 1),
            )
        # Output layout per row: [po0 (0:F) | den (F) | po1 (F+1:D+1)] —
        # the denominator sits between the halves so the final group's
        # split stores each depend on a single copy.
        if ib == NQ // P - 1:
            # Kernel-critical tail: c1 waits only po1's stop (the group's
            # 2nd-to-last PE tick), so its store half ships ~0.2us before
            # the denominator matmul even retires; the other half follows
            # its own copy.  High priority places the copies ahead of the
            # teardown drains on the DVE queue.  The split stores are
            # SWDGE descriptors 15 and 16, recycling the 8-sem pool's
            # sems of stores 7 and 8 — pre-observed on GpSimd so each
            # dma_start keeps a single data-ready wait.
            gg_b = nc.gpsimd.memset(gguard[0:1, ib + 1 : ib + 2], 0.0)
            tile.add_dep_helper(
                gg_b.ins, out_dmas[7].ins, reason="observe sem reuse on GpSimd"
            )
            with tc.high_priority():
                c1 = nc.vector.tensor_copy(out=o_sb[:, F + 1 : D + 1], in_=po1)
                tile.add_dep_helper(c1.ins, g2.ins, False, reason="order after g2")
            st_b = nc.gpsimd.dma_start(
                out=outr[ib][:, F + 1 : D + 1], in_=o_sb[:, F + 1 : D + 1]
            )
            gg_a = nc.gpsimd.memset(gguard[0:1, ib + 2 : ib + 3], 0.0)
            tile.add_dep_helper(
                gg_a.ins, out_dmas[8].ins, reason="observe sem reuse on GpSimd"
            )
            with tc.high_priority():
                dcp = nc.vector.tensor_copy(out=o_sb[:, F : F + 1], in_=pd[:, 0:1])
                tile.add_dep_helper(dcp.ins, c1.ins, False, reason="order after c1")
                c0 = nc.vector.tensor_copy(out=o_sb[:, 0:F], in_=po0)
                tile.add_dep_helper(c0.ins, dcp.ins, False, reason="order after dcp")
            st_a = nc.gpsimd.dma_start(
                out=outr[ib][:, 0 : F + 1], in_=o_sb[:, 0 : F + 1]
            )
            out_dmas.extend([st_b, st_a])
            last_cp = c0
            continue
        # Denominator copy first: pd's stop-matmul is the group's last
        # PE tick, so this copy's PE wait covers po0/po1 and the po
        # copies need only their (buffer-reuse) DVE wait.  The explicit
        # sync=False deps pin the scheduler to that order.
        dcp = nc.vector.tensor_copy(out=o_sb[:, F : F + 1], in_=pd[:, 0:1])
        tile.add_dep_helper(dcp.ins, g2.ins, False, reason="order after guard")
        c0 = nc.vector.tensor_copy(out=o_sb[:, 0:F], in_=po0)
        tile.add_dep_helper(c0.ins, dcp.ins, False, reason="order after dcp")
        last_cp = nc.vector.tensor_copy(out=o_sb[:, F + 1 : D + 1], in_=po1)
        tile.add_dep_helper(last_cp.ins, c0.ins, False, reason="order after c0")
        if ib >= 8:
            # SWDGE queue lap 2: pre-observe the queue's previous store on
            # GpSimd so the dma_start keeps its single data-ready wait.
            gg = nc.gpsimd.memset(gguard[0:1, ib : ib + 1], 0.0)
            tile.add_dep_helper(
                gg.ins, out_dmas[ib - 8].ins, reason="observe queue lap on GpSimd"
            )
        out_dmas.append(nc.gpsimd.dma_start(out=outr[ib], in_=o_sb))

    # Let SP observe every remaining proc's final tick so the auto-generated
    # kernel-tail drain needs no aggregated multi-sem wait of its own.
    for dd in out_dmas:
        sp_observe(dd, "observe output DMA on SP")
    sp_observe(last_exp, "observe ACT on SP")
    sp_observe(last_mm, "observe PE on SP")
    sp_observe(last_cp, "observe DVE on SP")


def build_attention_module():
    nc = bass.Bass(trn_type="TRN2", target_bir_lowering=False, debug=False)
    sizes = {
        "head": F + P, "wu0": 3 * P,
        "wu1": F, "xk1": F, "wv0": F, "wv1": F, "xq0": F, "xq1": F,
    }
    ins = {
        n: nc.dram_tensor(n, [P, DC * w], BF, kind="ExternalInput").ap()
        for n, w in sizes.items()
    }
    out = nc.dram_tensor("out", [NQ, D + 1], BF, kind="ExternalOutput").ap()
    with tile.TileContext(nc) as tc:
        with ExitStack() as ctx:
            _attention_kernel(ctx, tc, out, ins)
    return nc


_module_cache = None


def _get_module():
    global _module_cache
    if _module_cache is None:
        _module_cache = build_attention_module()
    return _module_cache


def _pc(a):
    """[D, M] -> [P, DC, M] (partition, chunk, col) view."""
    return a.reshape(DC, P, a.shape[1]).transpose(1, 0, 2)


def _pcf(a, lo, hi):
    """[D, M] fp32 -> [P, DC*(hi-lo)] bf16, [p, c, col] contiguous slice."""
    return (
        np.ascontiguousarray(_pc(a)[:, :, lo:hi])
        .reshape(P, DC * (hi - lo))
        .astype(ml_dtypes.bfloat16)
    )


def make_in_maps(x, Wq, Wk, Wv):
    x = np.asarray(x, dtype=np.float32)
    Wq = np.asarray(Wq, dtype=np.float32)
    Wk = np.asarray(Wk, dtype=np.float32)
    Wv = np.asarray(Wv, dtype=np.float32)
    # scores = q k^T = x (Wq^T Wk) x_k^T; fold G on the host.  The kernel's
    # stationary operand is wuT[d, e] = G[e, d] = (Wk^T Wq)[d, e].
    wu = Wk.T @ Wq
    wvT = np.ascontiguousarray(Wv.T)
    shared = {
        "wu0": _pcf(wu, P, 4 * P), "wu1": _pcf(wu, F, 2 * F),
        "wv0": _pcf(wvT, 0, F), "wv1": _pcf(wvT, F, 2 * F),
    }
    wu_e0 = _pc(wu)[:, :, 0:P]  # rides in the head transfer
    in_maps = []
    for core in range(NCORES):
        b, half = divmod(core, 2)
        xb = x[b]
        if half:
            xb = np.concatenate([xb[NKH:], xb[:NKH]], axis=0)
        xt = np.ascontiguousarray(xb.T)  # [D, N], key half first
        xk = xt[:, 0:NKH]
        xq = xt[:, NKH:]
        head = (
            np.concatenate([_pc(xk)[:, :, 0:F], wu_e0], axis=2)
            .reshape(P, DC * (F + P))
            .astype(ml_dtypes.bfloat16)
        )
        in_maps.append(
            {
                "head": head, "xk1": _pcf(xk, F, 2 * F),
                "xq0": _pcf(xq, 0, F), "xq1": _pcf(xq, F, 2 * F),
                **shared,
            }
        )
    return in_maps


def _install_ntff_hook_shim():
    """The container's `antenv` stub lacks axon_hooks; register an equivalent
    built on trn_agent_boot's ctypes NTFF driver so trace=True works."""
    import sys
    import types

    if "antenv.axon_hooks" in sys.modules:
        return
    try:
        from trn_agent_boot.trn_boot import _ntff_profile_via_ctypes

        hook = _ntff_profile_via_ctypes("/opt/axon/libaxon_pjrt.so")
    except Exception:
        hook = None
    mod = types.ModuleType("antenv.axon_hooks")
    mod.get_axon_ntff_profile_hook = lambda: hook
    sys.modules["antenv.axon_hooks"] = mod


def kernel(x, Wq, Wk, Wv, _trace=False, _trace_cores=None):
    if _trace:
        _install_ntff_hook_shim()
    in_maps = make_in_maps(x, Wq, Wk, Wv)
    nc = _get_module()
    res = run_bass_kernel_spmd(
        nc,
        in_maps,
        core_ids=list(range(NCORES)),
        trace=_trace,
        trace_cores=_trace_cores,
    )
    out = np.empty((B, N, D), dtype=np.float32)
    for b in range(B):
        r0 = np.asarray(res.results[2 * b]["out"], dtype=np.float32)
        r1 = np.asarray(res.results[2 * b + 1]["out"], dtype=np.float32)
        r1 = np.roll(r1, NKH, axis=0)  # undo the odd-core query rotation
        osum = r0 + r1
        den = osum[:, F : F + 1]  # denominator column sits between the halves
        out[b, :, 0:F] = osum[:, 0:F] / den
        out[b, :, F:D] = osum[:, F + 1 : D + 1] / den
    if _trace:
        return out, res
    return out
